# revision 1
# baseline (speedup 1.0000x reference)
"""Trainium2 Bass kernel for nn_ModelA_ViT: 8-layer ViT encoder (D=1024, 16 heads,
2D RoPE, RMSNorm, GELU-tanh MLP) over 4x16x64x64 input, output [4, 1024, 1024].

Sharding: sequence-parallel over (batch, token-half): core c owns batch c//2,
tokens [512*(c%2), 512*(c%2)+512). Per layer, each core computes q/k/v for its
512 tokens, all-gathers rope'd K and V across all 8 cores through DRAM, and
runs attention rows for its own tokens against the full 1024-token K/V of its
batch (shards selected with partition-id-based dynamic DMA offsets).

Layout: residual stream is [D on partitions (8x128), tokens on free (512)]
("layout B") end-to-end - matches the [N, D, L] output, so no transposes are
ever needed. RMSNorm per-token stats use a ones-vector matmul partition-reduce
plus a K=1 matmul broadcast. RoPE rotate-half is one DVE stream_shuffle.
Softmax runs without max-subtraction (scores are provably small for this
model); denominators come free from a ones-column appended to V's stationary
operand. All matmuls use float32r (full PE rate, fp32 PSUM accumulation).
"""

import sys

sys.path.insert(0, "/opt/trn_rl_repo")

import os

import numpy as np

D = 1024
HEADS = 16
DH = 64
DEPTH = int(os.environ.get("VIT_LAYERS", "8"))
HID = 4096
NB = 4
L = 1024
T = 512  # tokens per core
NCORES = 8
EPS = 1e-6
FACT = 2

_cache = {}

def _env(name, dflt):
    return int(os.environ.get(name, str(dflt)))

WG_B = _env("VIT_WG", 12)
ACC_B = _env("VIT_ACC", 4)
SC_B = _env("VIT_SC", 2)
P_B = _env("VIT_P", 3)
KST_B = _env("VIT_KST", 2)
SQ_B = _env("VIT_SQ", 3)
RT1_B = _env("VIT_RT1", 1)
RT2_B = _env("VIT_RT2", 1)
RT3_B = _env("VIT_RT3", 1)
OACC_B = _env("VIT_OACC", 2)
KF_B = _env("VIT_KF", 1)
H_B = _env("VIT_H", 1)


def _build():
    import concourse.bass as bass
    import concourse.bacc as bacc
    import concourse.mybir as mybir
    import concourse.tile as tile

    f32 = mybir.dt.float32
    f32r = mybir.dt.float32r
    AF = mybir.ActivationFunctionType
    ALU = mybir.AluOpType
    ds = bass.ds

    nc = bacc.Bacc("TRN2", target_bir_lowering=False, debug=False, num_devices=NCORES)

    patchesT_d = nc.dram_tensor("patchesT", [64, T], f32r, kind="ExternalInput")
    merge_wT_d = nc.dram_tensor("merge_wT", [64, D], f32r, kind="ExternalInput")
    qkvT_d = nc.dram_tensor("qkvT", [DEPTH, D, 3 * D], f32r, kind="ExternalInput")
    projT_d = nc.dram_tensor("projT", [DEPTH, D, D], f32r, kind="ExternalInput")
    fc1T_d = nc.dram_tensor("fc1T", [DEPTH, D, HID], f32r, kind="ExternalInput")
    fc2T_d = nc.dram_tensor("fc2T", [DEPTH, HID, D], f32r, kind="ExternalInput")
    emat2_d = nc.dram_tensor("emat2", [2, 128], f32r, kind="ExternalInput")
    cos_d = nc.dram_tensor("cos_t", [128, T], f32, kind="ExternalInput")
    sinm_d = nc.dram_tensor("sinm_t", [128, T], f32, kind="ExternalInput")
    z_d = nc.dram_tensor("z", [D, T], f32, kind="ExternalOutput")

    SHUF = list(range(16, 32)) + list(range(0, 16))
    NDT = D // 128  # 8 d-tiles
    NST = L // 128  # 8 kv s-tiles

    with tile.TileContext(nc) as tc:
        with (
            nc.allow_low_precision(reason="float32r matmul pipeline"),
            tc.tile_pool(name="const", bufs=1) as cpool,
            tc.tile_pool(name="sb", bufs=2) as sb,
            tc.tile_pool(name="ps", bufs=2, space="PSUM") as ps,
            tc.tile_pool(name="dram", bufs=1, space="DRAM") as dram,
        ):
            # ---- constants ----
            patches_sb = cpool.tile([64, T], f32r, name="patches_sb")
            nc.sync.dma_start(out=patches_sb, in_=patchesT_d[:, :])
            cos_sb = cpool.tile([128, T], f32, name="cos_sb")
            nc.sync.dma_start(out=cos_sb, in_=cos_d[:, :])
            sinm_sb = cpool.tile([128, T], f32, name="sinm_sb")
            nc.sync.dma_start(out=sinm_sb, in_=sinm_d[:, :])
            emat2_sb = cpool.tile([2, 128], f32r, name="emat2_sb")
            nc.sync.dma_start(out=emat2_sb, in_=emat2_d[:, :])
            ones_col = cpool.tile([128, 1], f32r, name="ones_col")
            nc.vector.memset(ones_col[:].bitcast(f32), 1.0)
            ones_row = cpool.tile([1, 128], f32r, name="ones_row")
            nc.vector.memset(ones_row[:].bitcast(f32), 1.0)
            eps_t = cpool.tile([1, 1], f32, name="eps_t")
            nc.vector.memset(eps_t, EPS)

            # ---- persistent tok tiles ----
            tok = [cpool.tile([128, T], f32, name=f"tok{i}") for i in range(NDT)]

            # partition-id derived shard row offsets in cc_out
            pid = nc.sync.partition_id()
            row0 = (pid // 2) * 4096  # even shard base row ([16384, 512] layout)
            row0h = (pid // 2) * 2048  # same in row-pair units

            # ---- layer 0 input: tok = merge_w @ patches ----
            for og in range(2):
                mw = sb.tile([64, 512], f32r, tag="wg", bufs=WG_B, name="mw")
                nc.sync.dma_start(out=mw, in_=merge_wT_d[:, 512 * og : 512 * (og + 1)])
                for j in range(4):
                    ot = 4 * og + j
                    acc = ps.tile([128, T], f32, tag="acc", bufs=ACC_B, name="m_acc")
                    nc.tensor.matmul(
                        acc, mw[:, 128 * j : 128 * (j + 1)], patches_sb[:],
                        start=True, stop=True,
                    )
                    nc.vector.tensor_copy(out=tok[ot], in_=acc)

            def rmsnorm_h(out_dtype=f32r, out_tag="h", bufs=H_B):
                """h = tok * rsqrt(mean(tok^2, d) + eps), all in layout B."""
                _sctag = "sc2" if os.environ.get("VIT_PAIREXP") == "1" else "sc"
                ssq = ps.tile([1, T], f32, tag=_sctag, bufs=SC_B, name="ssq")
                _sqeng = nc.gpsimd if os.environ.get("VIT_GPS") == "1" else nc.vector
                for dt in range(NDT):
                    sq = sb.tile([128, T], f32r, tag="sq", bufs=SQ_B, name="sq")
                    _sqeng.tensor_mul(sq, tok[dt], tok[dt])
                    nc.tensor.matmul(
                        ssq, ones_col[:], sq[:], start=(dt == 0), stop=(dt == NDT - 1)
                    )
                srow = sb.tile([1, T], f32, tag="srow", bufs=2, name="srow")
                nc.scalar.activation(
                    out=srow, in_=ssq, func=AF.Sqrt, bias=eps_t[:], scale=1.0 / D
                )
                rrow = sb.tile([1, T], f32r, tag="rrow", bufs=2, name="rrow")
                nc.vector.reciprocal(out=rrow, in_=srow)
                bc = ps.tile([128, T], f32, tag=_sctag, bufs=SC_B, name="bc")
                nc.tensor.matmul(bc, ones_row[:], rrow[:], start=True, stop=True)
                hs = []
                for dt in range(NDT):
                    ht = sb.tile([128, T], out_dtype, tag=f"{out_tag}{dt}", bufs=bufs,
                                 name="ht")
                    nc.vector.tensor_mul(ht, tok[dt], bc)
                    hs.append(ht)
                return hs

            def project(w_slice, act, n_ct, n_ot, out_cb):
                """out[ot] = sum_ct w_slice(ct, og)[:, j].T @ act[ct]; groups of 4."""
                wg_prev = []
                for og in range(n_ot // 4):
                    if os.environ.get("VIT_WONCE") == "1" and og > 0:
                        wg = wg_prev  # timing-only ablation: reuse stale weights
                    else:
                        wg = []
                        for ct in range(n_ct):
                            wt = sb.tile([128, 512], f32r, tag="wg", bufs=WG_B, name="wt")
                            nc.sync.dma_start(out=wt, in_=w_slice(ct, og))
                            wg.append(wt)
                        wg_prev = wg
                    for j in range(4):
                        acc = ps.tile([128, T], f32, tag="acc", bufs=ACC_B, name="p_acc")
                        for ct in range(n_ct):
                            nc.tensor.matmul(
                                acc, wg[ct][:, 128 * j : 128 * (j + 1)], act[ct][:],
                                start=(ct == 0), stop=(ct == n_ct - 1),
                            )
                        out_cb(og * 4 + j, acc)

            def rope(acc, dst):
                """dst = acc*cos + shuffle(acc)*sinm (dst f32r)."""
                t1 = sb.tile([128, T], f32, tag="rt1", bufs=RT1_B, name="rt1")
                nc.vector.stream_shuffle(out=t1[:], in_=acc[:], mask=SHUF)
                t2 = sb.tile([128, T], f32, tag="rt2", bufs=RT2_B, name="rt2")
                nc.vector.tensor_mul(t2, t1, sinm_sb)
                t3 = sb.tile([128, T], f32, tag="rt3", bufs=RT3_B, name="rt3")
                nc.vector.tensor_mul(t3, acc, cos_sb)
                nc.vector.tensor_add(dst, t2, t3)

            for lyr in range(DEPTH):
                # ---------- attention half ----------
                h = rmsnorm_h()

                cc_in = dram.tile([2048, 512], f32, tag="cci", bufs=2, name="cc_in")
                cc_out = dram.tile(
                    [NCORES * 2048, 512], f32,
                    addr_space="Local" if os.environ.get("VIT_NOCC") == "1" else "Shared",
                    bufs=1, name=f"cc_out{lyr}",
                )
                cc_v = cc_in[:].rearrange("(r two) c -> r two c", two=2)
                cco_v = cc_out[:].rearrange("(r two) c -> r two c", two=2)

                # k projection + rope + stage (cols [1024, 2048) of qkvT)
                def k_cb(ot, acc):
                    kt = sb.tile([128, T], f32r, tag="kst", bufs=KST_B, name="kt")
                    rope(acc, kt)
                    nc.sync.dma_start(
                        out=cc_in[128 * ot : 128 * (ot + 1), :],
                        in_=kt[:].bitcast(f32),
                    )

                project(
                    lambda ct, og: qkvT_d[
                        lyr, 128 * ct : 128 * (ct + 1), D + 512 * og : D + 512 * (og + 1)
                    ],
                    h, NDT, NDT, k_cb,
                )

                # v projection + stage: v[t, o] with t on partitions (layout A)
                for nch in range(2):
                    wv = []
                    for dt in range(NDT):
                        wt = sb.tile([128, 512], f32r, tag="wg", bufs=WG_B, name="wvt")
                        nc.sync.dma_start(
                            out=wt,
                            in_=qkvT_d[
                                lyr, 128 * dt : 128 * (dt + 1),
                                2 * D + 512 * nch : 2 * D + 512 * (nch + 1),
                            ],
                        )
                        wv.append(wt)
                    for tt in range(4):
                        acc = ps.tile([128, T], f32, tag="acc", bufs=ACC_B, name="v_acc")
                        for dt in range(NDT):
                            nc.tensor.matmul(
                                acc, h[dt][:, 128 * tt : 128 * (tt + 1)], wv[dt][:],
                                start=(dt == 0), stop=(dt == NDT - 1),
                            )
                        vt = sb.tile([128, T], f32r, tag="kst", bufs=KST_B, name="vt")
                        nc.vector.tensor_copy(out=vt, in_=acc)
                        nc.sync.dma_start(
                            out=cc_v[512 + 128 * tt : 512 + 128 * (tt + 1), nch, :],
                            in_=vt[:].bitcast(f32),
                        )

                if os.environ.get("VIT_NOCC") == "1":
                    # timing/simulation variant: fake the gather with local DMAs
                    nc.sync.dma_start(out=cc_out[0:2048, :], in_=cc_in[:])
                    nc.sync.dma_start(out=cc_out[2048:4096, :], in_=cc_in[:])
                else:
                    nc.gpsimd.collective_compute(
                        "AllGather",
                        ALU.bypass,
                        replica_groups=[list(range(NCORES))],
                        ins=[cc_in[:].opt()],
                        outs=[cc_out[:].opt()],
                    )

                # q projection + rope (overlaps the all-gather)
                q_tiles = [None] * NDT

                def q_cb(ot, acc):
                    qt = sb.tile([128, T], f32r, tag=f"q{ot}", bufs=1, name="qt")
                    rope(acc, qt)
                    q_tiles[ot] = qt

                def emit_q():
                    project(
                        lambda ct, og: qkvT_d[
                            lyr, 128 * ct : 128 * (ct + 1), 512 * og : 512 * (og + 1)
                        ],
                        h, NDT, NDT, q_cb,
                    )

                if os.environ.get("VIT_VFIRST", "1") != "1":
                    emit_q()

                # v readback into v_aug [128, 16*65] tiles (ones col per head)
                vf = []
                for st in range(NST):
                    vt = sb.tile([128, HEADS * 65], f32r, tag=f"vf{st}", bufs=1,
                                 name="vft")
                    vv = vt[:].rearrange("p (h j) -> p h j", j=65)
                    half_off = 0 if st < 4 else 1024  # odd shard, in row-pair units
                    for nch in range(2):
                        nc.sync.dma_start(
                            out=vv[:, 8 * nch : 8 * (nch + 1), 0:64].bitcast(f32),
                            in_=cco_v[
                                ds(row0h + half_off + 512 + 128 * (st % 4), 128),
                                nch, :,
                            ],
                        )
                    nc.vector.memset(vv[:, :, 64:65].bitcast(f32), 1.0)
                    vf.append(vt)

                if os.environ.get("VIT_VFIRST", "1") == "1":
                    emit_q()

                # attention per head-pair
                o_tiles = [None] * NDT
                if os.environ.get("VIT_NOATT") == "1":
                    o_tiles = h  # timing-only ablation
                for hp in range(NDT * (0 if os.environ.get("VIT_NOATT") == "1" else 1)):
                    kf0 = sb.tile([128, 512], f32r, tag="kf0", bufs=KF_B, name="kf0")
                    nc.sync.dma_start(
                        out=kf0[:].bitcast(f32),
                        in_=cc_out[ds(row0 + 128 * hp, 128), :],
                    )
                    kf1 = sb.tile([128, 512], f32r, tag="kf1", bufs=KF_B, name="kf1")
                    nc.sync.dma_start(
                        out=kf1[:].bitcast(f32),
                        in_=cc_out[ds(row0 + 2048 + 128 * hp, 128), :],
                    )
                    kfh = [kf0, kf1]
                    rd = sb.tile([1, 2 * T], f32r, tag="rr2", bufs=2, name="rd")
                    oaccs = []
                    if os.environ.get("VIT_PAIREXP") == "1":
                        oaccs = [
                            ps.tile([65, T], f32, tag="oacc", bufs=OACC_B,
                                    name=f"oacc{hh}")
                            for hh in range(2)
                        ]
                        for st in range(NST):
                            sc = ps.tile([128, 2 * T], f32, tag="sc2", bufs=SC_B,
                                         name="sc")
                            for hh in range(2):
                                nc.tensor.matmul(
                                    sc[:, T * hh : T * (hh + 1)],
                                    kfh[st // 4][
                                        64 * hh : 64 * hh + 64,
                                        128 * (st % 4) : 128 * (st % 4 + 1),
                                    ],
                                    q_tiles[hp][64 * hh : 64 * hh + 64, :],
                                    start=True, stop=True,
                                )
                            pt = sb.tile([128, 2 * T], f32r, tag="p", bufs=P_B,
                                         name="pt")
                            nc.scalar.activation(
                                out=pt, in_=sc, func=AF.Exp, scale=1.0 / np.sqrt(DH)
                            )
                            for hh in range(2):
                                nc.tensor.matmul(
                                    oaccs[hh],
                                    vf[st][:, 65 * (2 * hp + hh) : 65 * (2 * hp + hh) + 65],
                                    pt[:, T * hh : T * (hh + 1)],
                                    start=(st == 0), stop=(st == NST - 1),
                                )
                        for hh in range(2):
                            nc.vector.reciprocal(
                                out=rd[0:1, T * hh : T * (hh + 1)],
                                in_=oaccs[hh][64:65, :],
                            )
                    else:
                        for hh in range(2):
                            habs = 2 * hp + hh
                            oacc = ps.tile([65, T], f32, tag="oacc", bufs=OACC_B, name="oacc")
                            for st in range(NST):
                                sc = ps.tile([128, T], f32, tag="sc", bufs=SC_B, name="sc")
                                nc.tensor.matmul(
                                    sc,
                                    kfh[st // 4][
                                        64 * hh : 64 * hh + 64,
                                        128 * (st % 4) : 128 * (st % 4 + 1),
                                    ],
                                    q_tiles[hp][64 * hh : 64 * hh + 64, :],
                                    start=True, stop=True,
                                )
                                pt = sb.tile([128, T], f32r, tag="p", bufs=P_B, name="pt")
                                nc.scalar.activation(
                                    out=pt, in_=sc, func=AF.Exp, scale=1.0 / np.sqrt(DH)
                                )
                                nc.tensor.matmul(
                                    oacc,
                                    vf[st][:, 65 * habs : 65 * habs + 65],
                                    pt[:],
                                    start=(st == 0), stop=(st == NST - 1),
                                )
                            nc.vector.reciprocal(
                                out=rd[0:1, T * hh : T * (hh + 1)], in_=oacc[64:65, :]
                            )
                            oaccs.append(oacc)
                    rr_d = dram.tile([2, T], f32, tag="rrd", bufs=2, name="rr_d")
                    nc.sync.dma_start(
                        out=rr_d[:].rearrange("p t -> (p t)"),
                        in_=rd[0:1, 0 : 2 * T].bitcast(f32),
                    )
                    rr2 = sb.tile([2, T], f32r, tag="rr2b", bufs=2, name="rr2")
                    nc.sync.dma_start(out=rr2[:].bitcast(f32), in_=rr_d[:])
                    rb = ps.tile(
                        [128, T], f32,
                        tag="sc2" if os.environ.get("VIT_PAIREXP") == "1" else "sc",
                        bufs=SC_B, name="rb",
                    )
                    nc.tensor.matmul(rb, emat2_sb[:], rr2[:], start=True, stop=True)
                    rb_sb = sb.tile([128, T], f32, tag="rt2", bufs=RT2_B, name="rb_sb")
                    nc.vector.tensor_copy(out=rb_sb, in_=rb)
                    ot_t = sb.tile([128, T], f32r, tag=f"big{hp}", bufs=1, name="ot_t")
                    nc.vector.tensor_mul(
                        ot_t[0:64, :], oaccs[0][0:64, :], rb_sb[0:64, :]
                    )
                    nc.vector.tensor_mul(
                        ot_t[64:128, :], oaccs[1][0:64, :], rb_sb[64:128, :]
                    )
                    o_tiles[hp] = ot_t

                # proj + residual
                def proj_cb(dt, acc):
                    nc.vector.tensor_add(tok[dt], tok[dt], acc)

                project(
                    lambda ct, og: projT_d[
                        lyr, 128 * ct : 128 * (ct + 1), 512 * og : 512 * (og + 1)
                    ],
                    o_tiles, NDT, NDT, proj_cb,
                )

                # ---------- MLP half ----------
                if os.environ.get("VIT_NOMLP") == "1":
                    continue
                h2 = rmsnorm_h()
                delta = [None] * NDT
                for qr in range(4):
                    hid = [None] * 8

                    def fc1_cb(j, acc, qr=qr, hid=hid):
                        gt = sb.tile([128, T], f32r, tag=f"hid{j}", bufs=1, name="gt")
                        nc.scalar.activation(
                            out=gt, in_=acc, func=AF.Gelu_apprx_tanh
                        )
                        hid[j] = gt

                    project(
                        lambda ct, og, qr=qr: fc1T_d[
                            lyr, 128 * ct : 128 * (ct + 1),
                            1024 * qr + 512 * og : 1024 * qr + 512 * (og + 1),
                        ],
                        h2, NDT, 8, fc1_cb,
                    )

                    def fc2_cb(dt, acc, qr=qr, hid=hid):
                        if qr == 0:
                            dl = sb.tile([128, T], f32, tag=f"big{dt}", bufs=1,
                                         name="dl")
                            nc.vector.tensor_copy(out=dl, in_=acc)
                            delta[dt] = dl
                        else:
                            nc.vector.tensor_add(delta[dt], delta[dt], acc)
                            if qr == 3:
                                eng = (nc.gpsimd if os.environ.get("VIT_GPS") == "1"
                                       else nc.vector)
                                eng.tensor_add(tok[dt], tok[dt], delta[dt])

                    project(
                        lambda ct, og, qr=qr: fc2T_d[
                            lyr, 1024 * qr + 128 * ct : 1024 * qr + 128 * (ct + 1),
                            512 * og : 512 * (og + 1),
                        ],
                        hid, 8, NDT, fc2_cb,
                    )

            # final rmsnorm -> z
            zs = rmsnorm_h(out_dtype=f32, out_tag="zz", bufs=1)
            for dt in range(NDT):
                nc.sync.dma_start(
                    out=z_d[128 * dt : 128 * (dt + 1), :], in_=zs[dt][:]
                )

    nc.compile()
    return nc


def _host_inputs(x, merge_w, qkv_w, proj_w, fc1_w, fc2_w):
    x = np.asarray(x, np.float32)
    Hf = Wf = 32
    patches = (
        x.reshape(NB, 16, Hf, FACT, Wf, FACT)
        .transpose(0, 2, 4, 1, 3, 5)
        .reshape(NB, L, 16 * FACT * FACT)
    )
    qkvT = np.ascontiguousarray(np.asarray(qkv_w, np.float32).transpose(0, 2, 1))
    projT = np.ascontiguousarray(np.asarray(proj_w, np.float32).transpose(0, 2, 1))
    fc1T = np.ascontiguousarray(np.asarray(fc1_w, np.float32).transpose(0, 2, 1))
    fc2T = np.ascontiguousarray(np.asarray(fc2_w, np.float32).transpose(0, 2, 1))
    merge_wT = np.ascontiguousarray(np.asarray(merge_w, np.float32).T)

    # rope tables per token-half: rows r in [0,128) = 2 heads x 64 head-dims
    inv_freq = (1.0 / (10000.0 ** (np.arange(16, dtype=np.float32) / 16.0))).astype(
        np.float32
    )
    cos_t = np.empty((2, 128, T), np.float32)
    sinm_t = np.empty((2, 128, T), np.float32)
    for half in range(2):
        t_glob = np.arange(half * T, (half + 1) * T, dtype=np.float32)
        pos_h = np.floor(t_glob / Wf).astype(np.float32)
        pos_w = (t_glob % Wf).astype(np.float32)
        for r in range(128):
            rr = r % 64
            pos = pos_h if rr < 32 else pos_w
            j = rr % 16
            ang = pos * inv_freq[j]
            cos_t[half, r] = np.cos(ang)
            s = np.sin(ang)
            sinm_t[half, r] = -s if (rr % 32) < 16 else s

    emat2 = np.zeros((2, 128), np.float32)
    emat2[0, 0:64] = 1.0
    emat2[1, 64:128] = 1.0

    in_maps = []
    for c in range(NCORES):
        n, half = c // 2, c % 2
        patchesT = np.ascontiguousarray(patches[n, half * T : (half + 1) * T, :].T)
        in_maps.append(
            {
                "patchesT": patchesT,
                "merge_wT": merge_wT,
                "qkvT": qkvT[:DEPTH],
                "projT": projT[:DEPTH],
                "fc1T": fc1T[:DEPTH],
                "fc2T": fc2T[:DEPTH],
                "cos_t": np.ascontiguousarray(cos_t[half]),
                "sinm_t": np.ascontiguousarray(sinm_t[half]),
                "emat2": emat2,
            }
        )
    return in_maps


def kernel(
    x, merge_w, qkv_w, qkv_b, proj_w, proj_b, fc1_w, fc1_b, fc2_w, fc2_b
) -> np.ndarray:
    from concourse.bass_utils import run_bass_kernel_spmd

    # biases are structurally zero for this model; the device kernel omits them
    for b in (qkv_b, proj_b, fc1_b, fc2_b):
        assert not np.any(np.asarray(b)), "nonzero biases unsupported"

    if "nc" not in _cache:
        _cache["nc"] = _build()
    nc = _cache["nc"]

    in_maps = _host_inputs(x, merge_w, qkv_w, proj_w, fc1_w, fc2_w)
    res = run_bass_kernel_spmd(nc, in_maps, core_ids=list(range(NCORES)))
    z = np.empty((NB, D, L), np.float32)
    for c in range(NCORES):
        n, half = c // 2, c % 2
        z[n, :, half * T : (half + 1) * T] = res.results[c]["z"]
    return z

